# revision 7
# baseline (speedup 1.0000x reference)
"""Causal self-attention Trainium2 kernel (8 NeuronCores, residual fp8 DR).

Sharding: core c -> batch b = c//4, head group hg = c%4 (4 heads each).
Each core computes its heads' QKV projections, causal attention, and a
partial output projection yt[d, t] (transposed, bf16). Host sums the 4
partials per batch, transposes, and adds b_proj.

Precision strategy (gate: rel-err < 2e-2; this config emulates 1.2e-2):
  Every big GEMM runs fp8 with DoubleRow perf mode (0.5 cycles/row, 2x
  bf16 rate), with *residual correction*: operand A ~ A8(e4m3) +
  Ar(e5m2 of A-A8), so  A@B ~ A8@B8 + A8@Br + Ar@B8  (3 DR matmuls per
  2 contraction tiles = 0.75x bf16 PE cost for qkv/proj, and the
  correction terms reuse the already-quantized partner operand).
  Attention probabilities P stay raw e4m3 (their quantization error
  largely cancels between numerator and denominator of softmax); V is
  residual-corrected, so AV costs 0.5x bf16.  QK^T scores stay bf16
  (contraction 128 cannot double-pump).  exp bias -2 keeps P in fp8
  range; normalization cancels it.
Engine split: PE matmuls; ACT exp + half of proj copies; DVE qkv/mask/
recip/normalize/casts + other half.
Schedule: head h+1's QKV groups (PE-bound) are emitted between the
attention spans of head h (ACT-bound); output-proj chunks interleave
into the last head's spans.
"""
import numpy as np

B, S, D, H = 2, 2048, 2048, 16
HD = 128
NCORES = 8
HPC = H // (NCORES // B)     # heads per core = 4
NEG = -1e9
WS = 64.0                    # fp8 weight pre-scale
EXPB = -2.0                  # exp bias (cancelled by normalization)


def build_nc(S=S, D=D, nh=HPC, span=512):
    import concourse.bass as bass
    import concourse.mybir as mybir
    from concourse import bacc
    from concourse.tile import TileContext

    f32 = mybir.dt.float32
    bf16 = mybir.dt.bfloat16
    f8 = mybir.dt.float8e4
    f5 = mybir.dt.float8e5
    KT = D // 128          # contraction tiles for qkv
    TT = S // 128          # token tiles
    NS = S // span         # q spans
    KPS = span // 128      # k-blocks per span
    assert KT % 2 == 0 and KPS % 2 == 0 and nh % 2 == 0
    scale = float(HD) ** -0.5
    DR = mybir.MatmulPerfMode.DoubleRow

    nc = bacc.Bacc("TRN2", target_bir_lowering=False, debug=False)
    x8_d = nc.dram_tensor("xt8", [D, S], f8, kind="ExternalInput").ap()
    xr_d = nc.dram_tensor("xtr", [D, S], f5, kind="ExternalInput").ap()
    w8_d = nc.dram_tensor("wqkv8", [3 * nh * 128, D], f8,
                          kind="ExternalInput").ap()
    wr_d = nc.dram_tensor("wqkvr", [3 * nh * 128, D], f5,
                          kind="ExternalInput").ap()
    bq_d = nc.dram_tensor("bqkv", [128, 3 * nh], f32, kind="ExternalInput").ap()
    wp8_d = nc.dram_tensor("wproj8", [nh * 128, D], f8,
                           kind="ExternalInput").ap()
    wpr_d = nc.dram_tensor("wprojr", [nh * 128, D], f5,
                           kind="ExternalInput").ap()
    tm_d = nc.dram_tensor("trimaskT", [128, 128], f32, kind="ExternalInput").ap()
    id_d = nc.dram_tensor("identb", [128, 128], bf16, kind="ExternalInput").ap()
    oc_d = nc.dram_tensor("ones2", [128, 256], f8, kind="ExternalInput").ap()
    yt_d = nc.dram_tensor("yt", [D, S], bf16, kind="ExternalOutput").ap()

    Act = mybir.ActivationFunctionType
    Alu = mybir.AluOpType

    with TileContext(nc) as tc:
        from contextlib import ExitStack
        with ExitStack() as ctx:
            res = ctx.enter_context(tc.tile_pool(name="res", bufs=1))
            qk_p = ctx.enter_context(tc.tile_pool(name="qk", bufs=2))
            v_p = ctx.enter_context(tc.tile_pool(name="v", bufs=2))
            pt_p = ctx.enter_context(tc.tile_pool(name="pt", bufs=4))
            sm_p = ctx.enter_context(tc.tile_pool(name="sm", bufs=3))
            ob_p = ctx.enter_context(tc.tile_pool(name="ob", bufs=2))
            yst_p = ctx.enter_context(tc.tile_pool(name="yst", bufs=3))
            ps_mm = ctx.enter_context(
                tc.tile_pool(name="ps_mm", bufs=2, space="PSUM"))
            ps_st = ctx.enter_context(
                tc.tile_pool(name="ps_st", bufs=2, space="PSUM"))
            ps_av = ctx.enter_context(
                tc.tile_pool(name="ps_av", bufs=2, space="PSUM"))

            # constants (tiny; first on the queue)
            trimaskT = res.tile([128, 128], f32, tag="trimaskT")
            identb = res.tile([128, 128], bf16, tag="identb")
            ones2 = res.tile([128, 2, 128], f8, tag="ones2")
            bq = res.tile([128, 3 * nh], f32, tag="bq")
            expb = res.tile([128, 1], f32, tag="expb")
            nc.vector.memset(expb, EXPB)
            nc.sync.dma_start(trimaskT, tm_d)
            nc.sync.dma_start(identb, id_d)
            nc.sync.dma_start(ones2, oc_d)
            nc.sync.dma_start(bq, bq_d)

            # head-0 weight stripes first so qkv h0 isn't gated
            wq8 = [None] * (3 * nh)
            wqr = [None] * (3 * nh)

            def load_wq(hp):
                w8 = res.tile([128, KT, 128], f8, tag=f"wq8_{hp}",
                              name=f"wq8_{hp}")
                wr = res.tile([128, KT, 128], f5, tag=f"wqr_{hp}",
                              name=f"wqr_{hp}")
                nc.sync.dma_start(w8, w8_d[hp * 128:(hp + 1) * 128, :])
                nc.sync.dma_start(wr, wr_d[hp * 128:(hp + 1) * 128, :])
                wq8[hp], wqr[hp] = w8, wr

            for p in range(3):
                load_wq(p * nh + 0)

            # x stripes (main + residual), in column halves for fast start
            xT8 = res.tile([128, KT, S], f8, tag="xT8")
            xTr = res.tile([128, KT, S], f5, tag="xTr")
            hS = S // 2
            for hh in range(2):
                for kt in range(KT):
                    cs = slice(hh * hS, (hh + 1) * hS)
                    nc.sync.dma_start(xT8[:, kt, cs],
                                      x8_d[kt * 128:(kt + 1) * 128, cs])
                    nc.sync.dma_start(xTr[:, kt, cs],
                                      xr_d[kt * 128:(kt + 1) * 128, cs])

            for h in range(1, nh):
                for p in range(3):
                    load_wq(p * nh + h)

            wp8 = res.tile([128, nh, D], f8, tag="wp8")
            wpr = res.tile([128, nh, D], f5, tag="wpr")
            for h in range(nh):
                nc.sync.dma_start(wp8[:, h, :], wp8_d[h * 128:(h + 1) * 128, :])
                nc.sync.dma_start(wpr[:, h, :], wpr_d[h * 128:(h + 1) * 128, :])

            OT8 = res.tile([128, nh, S], f8, tag="OT8")
            OTr = res.tile([128, nh, S], f5, tag="OTr")

            def alloc_qkv_dsts(h):
                return [qk_p.tile([128, S], bf16, tag=("qt", "kt_", "vt")[p],
                                  name=f"{('qt', 'kt', 'vt')[p]}{h}")
                        for p in range(3)]

            def emit_qkv_group(h, p, spc, dsts):
                """One PSUM group pair: projection p, spans spc..spc+1.
                Per kt-pair: M1=W8@x8, M2=W8@xr, M3=Wr@x8 (all DR)."""
                hp = p * nh + h
                w8, wr = wq8[hp], wqr[hp]
                dst = dsts[p]
                nsp = min(2, NS - spc)
                pss = [ps_mm.tile([128, span], f32, tag="mm",
                                  name=f"mm{h}_{p}_{spc}_{i}")
                       for i in range(nsp)]
                NT = KT // 2
                for t in range(NT):
                    tsl = slice(2 * t, 2 * t + 2)
                    # same stationary tensor back-to-back across spans/terms
                    # (W8 x4, then Wr x2) to minimize PE weight reloads
                    for lhs, rhs in ((w8, xT8), (w8, xTr), (wr, xT8)):
                        for i in range(nsp):
                            sp = spc + i
                            csl = slice(sp * span, (sp + 1) * span)
                            st = (t == 0 and lhs is w8 and rhs is xT8)
                            en = (t == NT - 1 and lhs is wr)
                            nc.tensor.matmul(pss[i], lhs[:, tsl, :],
                                             rhs[:, tsl, csl],
                                             start=st, stop=en, perf_mode=DR)
                for i in range(nsp):
                    sp = spc + i
                    nc.vector.tensor_scalar(
                        out=dst[:, sp * span:(sp + 1) * span], in0=pss[i],
                        scalar1=1.0 / WS, scalar2=bq[:, hp:hp + 1],
                        op0=Alu.mult, op1=Alu.add)

            def emit_vtrans(h, VT, vh8, vhr):
                """Natural-layout V via PE transpose; split e4m3 + e5m2."""
                for tg in range(0, TT, 4):
                    n = min(4, TT - tg)
                    psf = ps_mm.tile([128, span], f32, tag="mm",
                                     name=f"tp{h}_{tg}")
                    pst = psf.bitcast(bf16)
                    for j in range(n):
                        nc.tensor.transpose(
                            pst[:, j * 128:(j + 1) * 128],
                            VT[:, (tg + j) * 128:(tg + j + 1) * 128], identb)
                    nc.vector.tensor_copy(vh8[:, tg:tg + n, :],
                                          pst[:, :n * 128])
                    nc.vector.tensor_tensor(
                        out=vhr[:, tg:tg + n, :], in0=pst[:, :n * 128],
                        in1=vh8[:, tg:tg + n, :], op=Alu.subtract)

            def emit_attention(h, sp, dsts, vh8, vhr):
                QT, KTt, _ = dsts
                npair = (KPS * (sp + 1)) // 2
                ps_o = ps_av.tile([128, span], f32, tag="o", name=f"o{h}_{sp}")
                ps_s = ps_av.tile([128, span], f32, tag="s", name=f"s{h}_{sp}")
                pend = []

                def flush_one():
                    m, pt, qoffp = pend.pop(0)
                    st, sp_ = (m == 0), (m == npair - 1)
                    nc.tensor.matmul(
                        ps_s[:, qoffp:], ones2, pt[:, :, qoffp:],
                        start=st, stop=sp_, perf_mode=DR)
                    nc.tensor.matmul(
                        ps_o[:, qoffp:], vh8[:, 2 * m:2 * m + 2, :],
                        pt[:, :, qoffp:],
                        start=st, stop=False, perf_mode=DR)
                    nc.tensor.matmul(
                        ps_o[:, qoffp:], vhr[:, 2 * m:2 * m + 2, :],
                        pt[:, :, qoffp:],
                        start=False, stop=sp_, perf_mode=DR)

                for m in range(npair):
                    qoffp = max(0, 2 * m - KPS * sp) * 128
                    pt = pt_p.tile([128, 2, span], f8, tag="pt",
                                   name=f"pt{h}_{sp}_{m}")
                    for i in range(2):
                        kj = 2 * m + i
                        own = max(0, kj - KPS * sp) * 128
                        ps = ps_st.tile([128, span], f32, tag="st",
                                        name=f"st{h}_{sp}_{kj}")
                        nc.tensor.matmul(
                            ps[:, own:], KTt[:, kj * 128:(kj + 1) * 128],
                            QT[:, sp * span + own:(sp + 1) * span],
                            start=True, stop=True)
                        if kj >= KPS * sp:  # diagonal block: causal mask
                            nc.vector.tensor_tensor(
                                out=ps[:, own:own + 128],
                                in0=ps[:, own:own + 128],
                                in1=trimaskT, op=Alu.add)
                        nc.scalar.activation(
                            pt[:, i, own:], ps[:, own:], Act.Exp,
                            bias=expb, scale=scale)
                        if own > qoffp:
                            nc.vector.memset(pt[:, i, qoffp:own], 0.0)
                    pend.append((m, pt, qoffp))
                    if len(pend) > 2:
                        flush_one()
                while pend:
                    flush_one()

                recipb = sm_p.tile([128, span], f32, tag="recipb",
                                   name=f"rb{h}_{sp}")
                nc.vector.reciprocal_approx_fast(out=recipb, in_=ps_s)
                csl = slice(sp * span, (sp + 1) * span)
                ob = ob_p.tile([128, span], bf16, tag="ob", name=f"ob{h}_{sp}")
                nc.vector.tensor_tensor(out=ob, in0=ps_o, in1=recipb,
                                        op=Alu.mult)
                nc.vector.tensor_copy(OT8[:, h, csl], ob)
                nc.vector.tensor_tensor(out=OTr[:, h, csl], in0=ob,
                                        in1=OT8[:, h, csl], op=Alu.subtract)

            def emit_proj_chunk(spc, dclo, dchi):
                nsp = min(2, NS - spc)
                for dc in range(dclo, dchi):
                    dsl = slice(dc * 128, (dc + 1) * 128)
                    pss = [ps_mm.tile([128, span], f32, tag="mm",
                                      name=f"pj{spc}_{dc}_{i}")
                           for i in range(nsp)]
                    NG = nh // 2
                    for g in range(NG):
                        gsl = slice(2 * g, 2 * g + 2)
                        for lhs, rhs in ((wp8, OT8), (wp8, OTr), (wpr, OT8)):
                            for i in range(nsp):
                                sp = spc + i
                                csl = slice(sp * span, (sp + 1) * span)
                                st = (g == 0 and lhs is wp8 and rhs is OT8)
                                en = (g == NG - 1 and lhs is wpr)
                                nc.tensor.matmul(
                                    pss[i], lhs[:, gsl, dsl], rhs[:, gsl, csl],
                                    start=st, stop=en, perf_mode=DR)
                    yst = yst_p.tile([128, nsp * span], bf16, tag="yst",
                                     name=f"yst{spc}_{dc}")
                    for i in range(nsp):
                        seg = yst[:, i * span:(i + 1) * span]
                        if dc % 2 == 0:
                            nc.scalar.mul(seg, pss[i], 1.0 / WS)
                        else:
                            nc.vector.tensor_scalar(
                                out=seg, in0=pss[i], scalar1=1.0 / WS,
                                scalar2=None, op0=Alu.mult)
                    nc.sync.dma_start(
                        yt_d[dc * 128:(dc + 1) * 128,
                             spc * span:(spc + nsp) * span], yst)

            # ---- software-pipelined schedule ----
            NDC = D // 128
            dsts = alloc_qkv_dsts(0)
            for spc in range(0, NS, 2):
                for p in range(3):
                    emit_qkv_group(0, p, spc, dsts)
            vh8 = v_p.tile([128, TT, 128], f8, tag="v8", name="vh8_0")
            vhr = v_p.tile([128, TT, 128], f5, tag="vr", name="vhr_0")
            emit_vtrans(0, dsts[2], vh8, vhr)

            for h in range(nh):
                fillers = [[] for _ in range(NS)]
                if h + 1 < nh:
                    nxt = alloc_qkv_dsts(h + 1)
                    nxtv8 = v_p.tile([128, TT, 128], f8, tag="v8",
                                     name=f"vh8_{h + 1}")
                    nxtvr = v_p.tile([128, TT, 128], f5, tag="vr",
                                     name=f"vhr_{h + 1}")
                    work = [(emit_qkv_group, (h + 1, p, spc, nxt))
                            for spc in range(0, NS, 2) for p in range(3)]
                    work.append((emit_vtrans, (h + 1, nxt[2], nxtv8, nxtvr)))
                    per = (len(work) + NS - 1) // NS
                    for sp in range(NS):
                        lo = sp * per
                        fillers[sp] = work[lo:lo + per] if sp < NS - 1 \
                            else work[lo:]
                else:
                    # output projection: chunk (spc, dc-range) becomes legal
                    # once span spc+nsp-1 of the last head is done
                    for spc in range(0, NS, 2):
                        nsp = min(2, NS - spc)
                        rdy = spc + nsp - 1
                        if rdy == NS - 1:   # final spans: emit whole chunk
                            fillers[rdy].append(
                                (emit_proj_chunk, (spc, 0, NDC)))
                        else:               # split across two spans
                            fillers[rdy].append(
                                (emit_proj_chunk, (spc, 0, NDC // 2)))
                            fillers[min(rdy + 1, NS - 1)].append(
                                (emit_proj_chunk, (spc, NDC // 2, NDC)))
                for sp in range(NS):
                    emit_attention(h, sp, dsts, vh8, vhr)
                    for fn, args in fillers[sp]:
                        fn(*args)
                if h + 1 < nh:
                    dsts, vh8, vhr = nxt, nxtv8, nxtvr

    nc.finalize()
    return nc


def _split48(a):
    """a (f32) -> (e4m3 main, e5m2 residual) as numpy arrays."""
    import ml_dtypes
    m = a.astype(ml_dtypes.float8_e4m3)
    r = (a - m.astype(np.float32)).astype(ml_dtypes.float8_e5m2)
    return m, r


def _prep_core_inputs(x, W_qkv, b_qkv, W_proj, core, S=S, D=D, nh=HPC,
                      b=None, hg=None):
    import ml_dtypes
    ngr = NCORES // B
    if b is None:
        b, hg = core // ngr, core % ngr
    KT = D // 128
    Dfull = W_qkv.shape[1] // 3

    wq = np.empty((3 * nh * 128, D), dtype=np.float32)
    bqt = np.zeros((128, 3 * nh), dtype=np.float32)
    for p in range(3):
        for h in range(nh):
            g = hg * nh + h
            col = p * Dfull + g * 128
            blk = W_qkv[:, col:col + 128]            # [D, 128]
            hp = p * nh + h
            wq[hp * 128:(hp + 1) * 128] = (
                blk.reshape(KT, 128, 128).transpose(1, 0, 2)
                .reshape(128, D) * WS)
            bqt[:, hp] = b_qkv[col:col + 128]
    wq8, wqr = _split48(wq)
    wp8, wpr = _split48(
        W_proj[hg * nh * 128:(hg + 1) * nh * 128, :].astype(np.float32) * WS)
    x8, xr = _split48(np.ascontiguousarray(x[b].T).astype(np.float32))

    r = np.arange(128)
    trimaskT = np.where(r[:, None] <= r[None, :], 0.0, NEG).astype(np.float32)
    return {
        "xt8": x8,
        "xtr": xr,
        "wqkv8": wq8,
        "wqkvr": wqr,
        "bqkv": bqt,
        "wproj8": wp8,
        "wprojr": wpr,
        "trimaskT": trimaskT,
        "identb": np.eye(128, dtype=ml_dtypes.bfloat16),
        "ones2": np.ones((128, 256), dtype=ml_dtypes.float8_e4m3),
    }


_CACHE = {}


def kernel(x, W_qkv, b_qkv, W_proj, b_proj, mask):
    from concourse.bass_utils import run_bass_kernel_spmd

    x = np.asarray(x)
    W_qkv = np.asarray(W_qkv)
    b_qkv = np.asarray(b_qkv)
    W_proj = np.asarray(W_proj)
    b_proj = np.asarray(b_proj)

    if "nc" not in _CACHE:
        _CACHE["nc"] = build_nc()
    nc = _CACHE["nc"]

    in_maps = [_prep_core_inputs(x, W_qkv, b_qkv, W_proj, c)
               for c in range(NCORES)]
    res = run_bass_kernel_spmd(nc, in_maps, core_ids=list(range(NCORES)))

    ngr = NCORES // B
    out = np.empty((B, S, D), dtype=np.float32)
    for b in range(B):
        acc = res.results[b * ngr]["yt"].astype(np.float32)
        for g in range(1, ngr):
            acc = acc + res.results[b * ngr + g]["yt"].astype(np.float32)
        out[b] = acc.T + b_proj[None, :]
    return out


# revision 11
# speedup vs baseline: 1.3738x; 1.3738x over previous
"""Causal self-attention Trainium2 kernel (8 NeuronCores, bf16 + fp8 P).

Sharding: core c -> batch b = c//4, head group hg = c%4 (4 heads each).
Each core computes its heads' QKV projections, causal attention, and a
partial output projection yt[d, t] (transposed, bf16). Host sums the 4
partials per batch, transposes, and adds b_proj.

Precision (gate 2e-2; this config ~1.2e-2): all GEMMs bf16 except the
attention probabilities P, which exp writes directly as fp8e4.  P's
quantization error largely cancels between the numerator (A@V) and
denominator (softmax sum) since both consume the SAME quantized P.
That makes the softmax-sum matmul eligible for fp8 DoubleRow (2x rate,
measured 1.04 cyc/col at 256-contraction when the pair stride is
>=2048B), and A@V runs as a mixed bf16xfp8 matmul at full bf16 rate
with V unquantized.  exp bias -2 keeps P in fp8 range (cancelled by
normalization).

PE work per core: qkv 164us + scores 29 + AV 29 + fp8 sum 16 + proj 55
~ 293us.  V's [tok,hd] copy runs on the DMA transpose XBAR, not PE.
Engine split: ACT exp + half proj copies; DVE qkv copies/mask/recip/
normalize + half proj copies.
Schedule: head h+1's QKV groups (PE-bound) interleave between the
attention spans of head h (ACT-bound); output-proj chunks interleave
into the last head's spans; head 0's QKV is emitted kt-major across 6
concurrent PSUM groups so the PE chases the x DMA stripe-by-stripe.
"""
import numpy as np

B, S, D, H = 2, 2048, 2048, 16
HD = 128
NCORES = 8
HPC = H // (NCORES // B)     # heads per core = 4
NEG = -1e9
EXPB = -2.0                  # exp bias (cancelled by normalization)
PTPAD = 2048                 # pt pair stride in bytes (>=2048 avoids the
                             # slow 2-stream DR path measured at stride 512)


def build_nc(S=S, D=D, nh=HPC, span=512):
    import concourse.bass as bass
    import concourse.mybir as mybir
    from concourse import bacc
    from concourse.tile import TileContext

    f32 = mybir.dt.float32
    bf16 = mybir.dt.bfloat16
    f8 = mybir.dt.float8e4
    KT = D // 128          # contraction tiles for qkv
    TT = S // 128          # token tiles
    NS = S // span         # q spans
    KPS = span // 128      # k-blocks per span
    assert KPS % 2 == 0
    scale = float(HD) ** -0.5
    DR = mybir.MatmulPerfMode.DoubleRow

    nc = bacc.Bacc("TRN2", target_bir_lowering=False, debug=False)
    x_d = nc.dram_tensor("xt", [D, S], bf16, kind="ExternalInput").ap()
    wq_d = nc.dram_tensor("wqkv", [3 * nh * 128, D], bf16,
                          kind="ExternalInput").ap()
    bq_d = nc.dram_tensor("bqkv", [128, 3 * nh], f32, kind="ExternalInput").ap()
    wp_d = nc.dram_tensor("wproj", [nh * 128, D], bf16,
                          kind="ExternalInput").ap()
    tm_d = nc.dram_tensor("trimaskT", [128, 128], f32, kind="ExternalInput").ap()
    oc_d = nc.dram_tensor("ones2", [128, 256], f8, kind="ExternalInput").ap()
    yt_d = nc.dram_tensor("yt", [D, S], bf16, kind="ExternalOutput").ap()

    Act = mybir.ActivationFunctionType
    Alu = mybir.AluOpType

    with TileContext(nc) as tc:
        from contextlib import ExitStack
        with ExitStack() as ctx:
            res = ctx.enter_context(tc.tile_pool(name="res", bufs=1))
            w_p = ctx.enter_context(tc.tile_pool(name="w", bufs=2))
            qk_p = ctx.enter_context(tc.tile_pool(name="qk", bufs=2))
            v_p = ctx.enter_context(tc.tile_pool(name="v", bufs=2))
            pt_p = ctx.enter_context(tc.tile_pool(name="pt", bufs=4))
            sm_p = ctx.enter_context(tc.tile_pool(name="sm", bufs=3))
            yst_p = ctx.enter_context(tc.tile_pool(name="yst", bufs=3))
            ps_mm = ctx.enter_context(
                tc.tile_pool(name="ps_mm", bufs=2, space="PSUM"))
            ps_st = ctx.enter_context(
                tc.tile_pool(name="ps_st", bufs=2, space="PSUM"))
            ps_av = ctx.enter_context(
                tc.tile_pool(name="ps_av", bufs=2, space="PSUM"))

            # constants (tiny; first on the queue)
            trimaskT = res.tile([128, 128], f32, tag="trimaskT")
            ones2 = res.tile([128, 2, 128], f8, tag="ones2")
            bq = res.tile([128, 3 * nh], f32, tag="bq")
            expb = res.tile([128, 1], f32, tag="expb")
            nc.vector.memset(expb, EXPB)
            nc.sync.dma_start(trimaskT, tm_d)
            nc.sync.dma_start(ones2, oc_d)
            nc.sync.dma_start(bq, bq_d)

            wq = {}

            def load_wq(h):
                for p in range(3):
                    hp = p * nh + h
                    w = w_p.tile([128, KT, 128], bf16, tag=f"w{p}",
                                 name=f"wq{hp}")
                    nc.sync.dma_start(w, wq_d[hp * 128:(hp + 1) * 128, :])
                    wq[hp] = w

            load_wq(0)

            # x stripes
            xT = res.tile([128, KT, S], bf16, tag="xT")
            for kt in range(KT):
                nc.sync.dma_start(xT[:, kt, :], x_d[kt * 128:(kt + 1) * 128, :])

            wp3 = res.tile([128, nh, D], bf16, tag="wp3")
            for h in range(nh):
                nc.sync.dma_start(wp3[:, h, :], wp_d[h * 128:(h + 1) * 128, :])

            OT = res.tile([128, nh, S], bf16, tag="OT")

            def alloc_qkv_dsts(h):
                return [qk_p.tile([128, S], bf16, tag=("qt", "kt_", "vt")[p],
                                  name=f"{('qt', 'kt', 'vt')[p]}{h}")
                        for p in range(3)]

            def qkv_copyout(h, p, pss, spc, nsp, dst):
                for i in range(nsp):
                    sp = spc + i
                    nc.vector.tensor_scalar(
                        out=dst[:, sp * span:(sp + 1) * span], in0=pss[i],
                        scalar1=bq[:, (p * nh + h):(p * nh + h) + 1],
                        scalar2=None, op0=Alu.add)

            def emit_qkv_group(h, p, spc, dsts):
                """One PSUM group pair: projection p, spans spc..spc+1."""
                hp = p * nh + h
                w = wq[hp]
                nsp = min(2, NS - spc)
                pss = [ps_mm.tile([128, span], f32, tag="mm",
                                  name=f"mm{h}_{p}_{spc}_{i}")
                       for i in range(nsp)]
                for kt in range(KT):
                    for i in range(nsp):
                        sp = spc + i
                        nc.tensor.matmul(
                            pss[i], w[:, kt, :],
                            xT[:, kt, sp * span:(sp + 1) * span],
                            start=(kt == 0), stop=(kt == KT - 1))
                qkv_copyout(h, p, pss, spc, nsp, dsts[p])

            def emit_qkv_head0(dsts):
                """Head 0 qkv, kt-major across 6 concurrent PSUM groups so
                the PE can start as soon as the first x stripes land."""
                if NS < 2:
                    for spc in range(0, NS, 2):
                        for p in range(3):
                            emit_qkv_group(0, p, spc, dsts)
                    return
                pss = {}
                for p in range(3):
                    for i in range(2):
                        pools_i = [ps_mm, ps_st, ps_av][p]
                        pss[(p, i)] = pools_i.tile(
                            [128, span], f32, tag=["mm", "st", "o"][p] if p < 2
                            else ("o" if i == 0 else "s"),
                            name=f"h0mm{p}_{i}")
                for kt in range(KT):
                    for p in range(3):
                        w = wq[p * nh]
                        for i in range(2):
                            nc.tensor.matmul(
                                pss[(p, i)], w[:, kt, :],
                                xT[:, kt, i * span:(i + 1) * span],
                                start=(kt == 0), stop=(kt == KT - 1))
                for p in range(3):
                    qkv_copyout(0, p, [pss[(p, 0)], pss[(p, 1)]], 0, 2,
                                dsts[p])
                for spc in range(2, NS, 2):
                    for p in range(3):
                        emit_qkv_group(0, p, spc, dsts)

            def emit_vtrans(h, VT, vh):
                """vh[128, TT, 128] = natural-layout V via DMA transpose."""
                for blk in range(TT):
                    nc.sync.dma_start(vh[:, blk, :],
                                      VT[:, blk * 128:(blk + 1) * 128],
                                      transpose=True)

            def emit_attention(h, sp, dsts, vh):
                QT, KTt, _ = dsts
                nkj = KPS * (sp + 1)
                npair = nkj // 2
                ps_o = ps_av.tile([128, span], f32, tag="o", name=f"o{h}_{sp}")
                ps_s = ps_av.tile([128, span], f32, tag="s", name=f"s{h}_{sp}")
                pend = []

                def flush_one():
                    kj, pt, own, qoffp = pend.pop(0)
                    # A@V: mixed bf16 x fp8, per k-block, exact causal cols
                    nc.tensor.matmul(
                        ps_o[:, own:span], vh[:, kj, :],
                        pt[:, kj % 2, own:span],
                        start=(kj == 0), stop=(kj == nkj - 1))
                    if kj % 2 == 1:
                        # softmax denominator: fp8 DoubleRow over the pair
                        m = kj // 2
                        nc.tensor.matmul(
                            ps_s[:, qoffp:span], ones2, pt[:, :, qoffp:span],
                            start=(m == 0), stop=(m == npair - 1),
                            perf_mode=DR)

                for kj in range(nkj):
                    own = max(0, kj - KPS * sp) * 128
                    qoffp = max(0, (kj - kj % 2) - KPS * sp) * 128
                    if kj % 2 == 0:
                        pt = pt_p.tile([128, 2, PTPAD], f8, tag="pt",
                                       name=f"pt{h}_{sp}_{kj // 2}")
                    else:
                        pt = pend[-1][1]
                    ps = ps_st.tile([128, span], f32, tag="st",
                                    name=f"st{h}_{sp}_{kj}")
                    nc.tensor.matmul(
                        ps[:, own:], KTt[:, kj * 128:(kj + 1) * 128],
                        QT[:, sp * span + own:(sp + 1) * span],
                        start=True, stop=True)
                    if kj >= KPS * sp:  # diagonal block: causal mask
                        nc.vector.tensor_tensor(
                            out=ps[:, own:own + 128],
                            in0=ps[:, own:own + 128],
                            in1=trimaskT, op=Alu.add)
                    nc.scalar.activation(
                        pt[:, kj % 2, own:span], ps[:, own:], Act.Exp,
                        bias=expb, scale=scale)
                    if own > qoffp:   # zero the odd member's pre-diag block
                        nc.vector.memset(pt[:, kj % 2, qoffp:own], 0.0)
                    pend.append((kj, pt, own, qoffp))
                    if len(pend) > 4:
                        flush_one()
                while pend:
                    flush_one()

                recipb = sm_p.tile([128, span], f32, tag="recipb",
                                   name=f"rb{h}_{sp}")
                nc.vector.reciprocal_approx_fast(out=recipb, in_=ps_s)
                nc.vector.tensor_tensor(
                    out=OT[:, h, sp * span:(sp + 1) * span],
                    in0=ps_o, in1=recipb, op=Alu.mult)

            def emit_proj_chunk(spc, dclo, dchi):
                nsp = min(2, NS - spc)
                for dc in range(dclo, dchi):
                    dsl = slice(dc * 128, (dc + 1) * 128)
                    pss = [ps_mm.tile([128, span], f32, tag="mm",
                                      name=f"pj{spc}_{dc}_{i}")
                           for i in range(nsp)]
                    for hh in range(nh):
                        for i in range(nsp):
                            sp = spc + i
                            nc.tensor.matmul(
                                pss[i], wp3[:, hh, dsl],
                                OT[:, hh, sp * span:(sp + 1) * span],
                                start=(hh == 0), stop=(hh == nh - 1))
                    yst = yst_p.tile([128, nsp * span], bf16, tag="yst",
                                     name=f"yst{spc}_{dc}")
                    for i in range(nsp):
                        seg = yst[:, i * span:(i + 1) * span]
                        if dc % 2 == 0:
                            nc.scalar.copy(seg, pss[i])
                        else:
                            nc.vector.tensor_copy(seg, pss[i])
                    nc.sync.dma_start(
                        yt_d[dc * 128:(dc + 1) * 128,
                             spc * span:(spc + nsp) * span], yst)

            # ---- software-pipelined schedule ----
            NDC = D // 128
            dsts = alloc_qkv_dsts(0)
            emit_qkv_head0(dsts)
            vh = v_p.tile([128, TT, 128], bf16, tag="v", name="vh0")
            emit_vtrans(0, dsts[2], vh)

            for h in range(nh):
                fillers = [[] for _ in range(NS)]
                if h + 1 < nh:
                    load_wq(h + 1)
                    nxt = alloc_qkv_dsts(h + 1)
                    nxtvh = v_p.tile([128, TT, 128], bf16, tag="v",
                                     name=f"vh{h + 1}")
                    work = [(emit_qkv_group, (h + 1, p, spc, nxt))
                            for spc in range(0, NS, 2) for p in range(3)]
                    work.append((emit_vtrans, (h + 1, nxt[2], nxtvh)))
                    per = (len(work) + NS - 1) // NS
                    for sp in range(NS):
                        lo = sp * per
                        fillers[sp] = work[lo:lo + per] if sp < NS - 1 \
                            else work[lo:]
                else:
                    # output projection: chunk (spc, dc-range) becomes legal
                    # once span spc+nsp-1 of the last head is done
                    for spc in range(0, NS, 2):
                        nsp = min(2, NS - spc)
                        rdy = spc + nsp - 1
                        if rdy == NS - 1:   # final spans: emit whole chunk
                            fillers[rdy].append(
                                (emit_proj_chunk, (spc, 0, NDC)))
                        else:               # split across two spans
                            fillers[rdy].append(
                                (emit_proj_chunk, (spc, 0, NDC // 2)))
                            fillers[min(rdy + 1, NS - 1)].append(
                                (emit_proj_chunk, (spc, NDC // 2, NDC)))
                for sp in range(NS):
                    emit_attention(h, sp, dsts, vh)
                    for fn, args in fillers[sp]:
                        fn(*args)
                if h + 1 < nh:
                    dsts, vh = nxt, nxtvh

    nc.finalize()
    return nc


def _prep_core_inputs(x, W_qkv, b_qkv, W_proj, core, S=S, D=D, nh=HPC,
                      b=None, hg=None):
    import ml_dtypes
    bf16 = ml_dtypes.bfloat16
    ngr = NCORES // B
    if b is None:
        b, hg = core // ngr, core % ngr
    KT = D // 128
    Dfull = W_qkv.shape[1] // 3

    wq = np.empty((3 * nh * 128, D), dtype=bf16)
    bqt = np.zeros((128, 3 * nh), dtype=np.float32)
    for p in range(3):
        for h in range(nh):
            g = hg * nh + h
            col = p * Dfull + g * 128
            blk = W_qkv[:, col:col + 128]            # [D, 128]
            hp = p * nh + h
            wq[hp * 128:(hp + 1) * 128] = (
                blk.reshape(KT, 128, 128).transpose(1, 0, 2).reshape(128, D)
                .astype(bf16))
            bqt[:, hp] = b_qkv[col:col + 128]
    wp = W_proj[hg * nh * 128:(hg + 1) * nh * 128, :].astype(bf16)

    r = np.arange(128)
    trimaskT = np.where(r[:, None] <= r[None, :], 0.0, NEG).astype(np.float32)
    return {
        "xt": np.ascontiguousarray(x[b].T).astype(bf16),
        "wqkv": wq,
        "bqkv": bqt,
        "wproj": wp,
        "trimaskT": trimaskT,
        "ones2": np.ones((128, 256), dtype=ml_dtypes.float8_e4m3),
    }


_CACHE = {}


def kernel(x, W_qkv, b_qkv, W_proj, b_proj, mask):
    from concourse.bass_utils import run_bass_kernel_spmd

    x = np.asarray(x)
    W_qkv = np.asarray(W_qkv)
    b_qkv = np.asarray(b_qkv)
    W_proj = np.asarray(W_proj)
    b_proj = np.asarray(b_proj)

    if "nc" not in _CACHE:
        _CACHE["nc"] = build_nc()
    nc = _CACHE["nc"]

    in_maps = [_prep_core_inputs(x, W_qkv, b_qkv, W_proj, c)
               for c in range(NCORES)]
    res = run_bass_kernel_spmd(nc, in_maps, core_ids=list(range(NCORES)))

    ngr = NCORES // B
    out = np.empty((B, S, D), dtype=np.float32)
    for b in range(B):
        acc = res.results[b * ngr]["yt"].astype(np.float32)
        for g in range(1, ngr):
            acc = acc + res.results[b * ngr + g]["yt"].astype(np.float32)
        out[b] = acc.T + b_proj[None, :]
    return out
